# revision 3
# baseline (speedup 1.0000x reference)
"""Trainium2 Bass kernel for nn_CoreGroupConstruction (segment_reduce).

Reference computation (see problem): a [8192, 2048] x [2048, 2048] masked
softmax-weighted reduction S = Wm @ (seed * exp(P)), followed by a
bernoulli log-likelihood over all (edge, node) pairs plus degree/size
moment-matching losses on the row/column sums of S.

Strategy (matches the sharding hint):
 - Host precomputes the tiny edge-independent pieces in f64: theta =
   log_sigmoid(theta_log), P [2048,2048] (4 matvec-sized matmuls over K=32),
   seed = softmax(seed_prob), E' = seed[:,None]*exp(P), and the row-normalized
   seed weights Wm. These are O(NC^2) with trivial flops; E'/Wm are shipped
   to the device in bf16.
 - The edge dimension M=8192 is sharded across the 8 cores (1024 edges each).
   Each core computes S = Wm_c @ E' (bf16 matmul, f32 PSUM accumulate), the
   pointwise loss sum -sum log(mask*S + (1-mask)*(1-S)) via the identity
   B = m2*S + b with m2 = 2*mask-1, b = 1-mask (one DVE mul + add, then one
   ACT Ln pass with fused per-partition accumulation), row sums of S
   (size_exp partials) and a running column accumulation of S (degree_exp
   partials).
 - Host gathers: sums the per-core loss partials and degree partials in f64
   (the "all-reduce" of the hint), concatenates size partials, sorts the
   [2048]/[8192] vectors, and assembles the final scalar.
"""

import numpy as np
import ml_dtypes

import concourse.bacc as bacc
import concourse.tile as tile
from concourse import mybir
from concourse.bass_utils import run_bass_kernel_spmd

M, NC, K = 8192, 2048, 32
N_CORES = 8
MLOC = M // N_CORES          # 1024 edges per core
P_DIM = 128
ET = MLOC // P_DIM           # 8 edge tiles per core
IC = NC // P_DIM             # 16 contraction chunks
JBLK = 512                   # one f32 PSUM bank
NJ = NC // JBLK              # 4
EB_GROUPS = 4                # E' split into 4 resident tiles (load/compute overlap)

_BF16 = ml_dtypes.bfloat16

_cache = {}


def _build_bass():
    nc = bacc.Bacc("TRN2", target_bir_lowering=False, debug=False)
    bf16 = mybir.dt.bfloat16
    f32 = mybir.dt.float32

    eb_d = nc.dram_tensor("eb", [P_DIM, IC, NC], bf16, kind="ExternalInput")
    wm_d = nc.dram_tensor("wm", [ET, P_DIM, IC, P_DIM], bf16, kind="ExternalInput")
    m2_d = nc.dram_tensor("m2", [ET, P_DIM, NC], bf16, kind="ExternalInput")
    bb_d = nc.dram_tensor("bb", [ET, P_DIM, NC], bf16, kind="ExternalInput")
    loss_d = nc.dram_tensor("loss_pp", [P_DIM, ET], f32, kind="ExternalOutput")
    deg_d = nc.dram_tensor("deg", [P_DIM, NC], f32, kind="ExternalOutput")
    sz_d = nc.dram_tensor("sizes", [P_DIM, ET], f32, kind="ExternalOutput")

    gsz = IC // EB_GROUPS
    with tile.TileContext(nc) as tc:
        with (
            tc.tile_pool(name="const", bufs=1) as cpool,
            tc.tile_pool(name="wmp", bufs=2) as wpool,
            tc.tile_pool(name="mbp", bufs=2) as mbpool,
            tc.tile_pool(name="work", bufs=2) as workpool,
            tc.tile_pool(name="psum", bufs=2, space="PSUM") as pspool,
        ):
            eb_tiles = []
            for g in range(EB_GROUPS):
                t = cpool.tile([P_DIM, gsz, NC], bf16, tag=f"eb{g}")
                nc.sync.dma_start(t[:], eb_d[:, g * gsz:(g + 1) * gsz, :])
                eb_tiles.append(t)
            loss_pp = cpool.tile([P_DIM, ET], f32, tag="loss")
            sizes_t = cpool.tile([P_DIM, ET], f32, tag="sizes")
            deg_acc = cpool.tile([P_DIM, NC], f32, tag="deg")
            nc.vector.memset(deg_acc[:], 0.0)

            for et in range(ET):
                wm_t = wpool.tile([P_DIM, IC, P_DIM], bf16, tag="wm")
                nc.sync.dma_start(wm_t[:], wm_d[et])
                m2_t = mbpool.tile([P_DIM, NC], bf16, tag="m2")
                nc.sync.dma_start(m2_t[:], m2_d[et])
                bb_t = mbpool.tile([P_DIM, NC], bf16, tag="bb")
                nc.sync.dma_start(bb_t[:], bb_d[et])

                ps = pspool.tile([P_DIM, NC], f32, tag="ps")
                for ic in range(IC):
                    lhsT = wm_t[:, ic, :]
                    rhs_t = eb_tiles[ic // gsz]
                    for jc in range(NJ):
                        nc.tensor.matmul(
                            ps[:, jc * JBLK:(jc + 1) * JBLK],
                            lhsT,
                            rhs_t[:, ic % gsz, jc * JBLK:(jc + 1) * JBLK],
                            start=(ic == 0),
                            stop=(ic == IC - 1),
                        )

                # size_exp partials: row sums of S
                nc.vector.reduce_sum(
                    sizes_t[:, et:et + 1], ps[:], axis=mybir.AxisListType.X
                )
                # degree_exp partials: running column accumulation of S
                nc.vector.tensor_add(deg_acc[:], deg_acc[:], ps[:])
                # B = mask*S + (1-mask)*(1-S) = m2*S + b
                b_t = workpool.tile([P_DIM, NC], f32, tag="B")
                nc.vector.tensor_mul(b_t[:], ps[:], m2_t[:])
                nc.vector.tensor_add(b_t[:], b_t[:], bb_t[:])
                # loss partial: sum_j ln(B) per partition (fused accumulate)
                scr = workpool.tile([P_DIM, NC], f32, tag="scr")
                nc.scalar.activation(
                    scr[:], b_t[:], mybir.ActivationFunctionType.Ln,
                    accum_out=loss_pp[:, et:et + 1],
                )

            nc.sync.dma_start(loss_d[:], loss_pp[:])
            nc.sync.dma_start(sz_d[:], sizes_t[:])
            nc.sync.dma_start(deg_d[:], deg_acc[:])
    nc.compile()
    return nc


def _host_precompute(theta_log, seed_prob, Ic, c2a):
    theta = -np.logaddexp(0.0, -theta_log.astype(np.float64))  # log_sigmoid [K,3]
    A = c2a.astype(np.float64)
    nA = 1.0 - A
    t0, t1, t2 = theta[:, 0], theta[:, 1], theta[:, 2]
    P = (nA * t0) @ nA.T + (A * t1) @ nA.T + (nA * t1) @ A.T + (A * t2) @ A.T
    np.fill_diagonal(P, 0.0)
    sp = seed_prob.astype(np.float64)
    seed = np.exp(sp - sp.max())
    seed /= seed.sum()
    Eprime = np.exp(P)                           # [NC, NC]
    Icf = Ic.astype(np.float64)
    rs = Icf @ seed                              # [M]
    Wm = (Icf * seed[None, :]) / rs[:, None]     # [M, NC]
    return Eprime, Wm, Icf


def kernel(theta_log, seed_prob, Ic, c2a):
    assert Ic.shape == (M, NC) and c2a.shape == (NC, K)
    Eprime, Wm, Icf = _host_precompute(theta_log, seed_prob, Ic, c2a)

    # device operand layouts (all contiguous for single-descriptor DMAs)
    eb_np = np.ascontiguousarray(
        Eprime.reshape(IC, P_DIM, NC).transpose(1, 0, 2)
    ).astype(_BF16)                              # eb[p, ic, j] = E'[ic*128+p, j]
    in_maps = []
    for c in range(N_CORES):
        sl = slice(c * MLOC, (c + 1) * MLOC)
        Wc = Wm[sl]                              # [1024, 2048]
        # wm[et, p, ic, el] = Wm_c[et*128+el, ic*128+p]  (lhsT layout)
        wm_np = np.ascontiguousarray(
            Wc.reshape(ET, P_DIM, IC, P_DIM).transpose(0, 3, 2, 1)
        ).astype(_BF16)
        Icc = Ic[sl].astype(np.float32)
        m2_np = (2.0 * Icc - 1.0).reshape(ET, P_DIM, NC).astype(_BF16)
        bb_np = (1.0 - Icc).reshape(ET, P_DIM, NC).astype(_BF16)
        in_maps.append({"eb": eb_np, "wm": wm_np, "m2": m2_np, "bb": bb_np})

    if "nc" not in _cache:
        _cache["nc"] = _build_bass()
    res = run_bass_kernel_spmd(_cache["nc"], in_maps, core_ids=list(range(N_CORES)))

    loss_sum = 0.0
    deg = np.zeros(NC, dtype=np.float64)
    sizes = np.zeros(M, dtype=np.float64)
    for c, r in enumerate(res.results):
        loss_sum += r["loss_pp"].astype(np.float64).sum()
        deg += r["deg"].astype(np.float64).sum(axis=0)
        sizes[c * MLOC:(c + 1) * MLOC] = r["sizes"].astype(np.float64).T.reshape(MLOC)

    loss = -loss_sum
    degree_exp = np.sort(deg)[::-1]
    size_exp = np.sort(sizes)[::-1]
    degree_ans = np.sort(Icf.sum(axis=0))[::-1]
    size_ans = np.sort(Icf.sum(axis=1))[::-1]
    degree_loss = np.mean((degree_exp - degree_ans) ** 2)
    size_loss = np.mean((size_exp - size_ans) ** 2)
    return np.float32(loss + degree_loss + size_loss)
